# revision 26
# baseline (speedup 1.0000x reference)
# Transformer-XL style relative-position attention on 8 Trainium2 NeuronCores.
#
# Contract: kernel(**inputs) takes the FULL unsharded inputs and returns the
# FULL [8, 256, 1024] output. Internally shards data-parallel over batch:
# core b computes batch element b. No collectives needed.
#
# Math (per batch element):
#   cat = [h; x]                            [512, 1024]
#   q,k,v = split(cat @ Wqkv)               heads=16, dhead=64
#   RW    = R @ Wkr                         [1024, 1024] (relative pos keys)
#   dots  = (q+u) @ k^T + rel_shift((q+v) @ RW_h^T)
#   out   = softmax(dots*8^-1 + causal/mem band mask) @ v @ Wout
#
# Key design points (v2 — restructured for load/compute overlap):
#  * Only 257 rows of RW are ever needed (offsets s in [256, 512]).
#  * rel_shift via DRAM scratch shear: write the [128, 258] BD band to a
#    [128, 767] scratch row and read back with AP [[766, 128], [1, 384]],
#    which realizes band[i, j] = BD[i, j - i + c]. Scratch pre-filled with
#    NEG outside the band so the read returns band+mask in one tensor.
#  * The whole schedule is arranged around the HBM load stream (24 MB of
#    f32 weights at ~370 GB/s ~= 62 us): projections run dt-outer in
#    ft-pairs so they track the arrival of each weight tile; the 32 BD
#    matmuls + band writes flood the PE/sync queues while Wk streams in;
#    the A/exp/transpose pipeline runs while Wv/Wout stream; AV + out
#    projection form the tail.
#  * Band writes and reads are split across the two HWDGE rings (sync and
#    scalar) to halve per-ring serialization.
#  * Attention inner loop is software-pipelined with explicit lookahead so
#    no engine FIFO head-blocks on the DRAM shear round trip.
#  * Normalization 1/S rides the mandatory exp transpose via diag(1/S).
#  * All matmul operands fp16; psum f32 except the exp transposes (f16).

import numpy as np

import concourse.bass as bass
import concourse.mybir as mybir
import concourse.tile as tile
from concourse import bacc, bass_utils
from concourse.masks import make_identity
from concourse.tile import add_dep_helper
from contextlib import ExitStack

F32 = mybir.dt.float32
F16 = mybir.dt.float16
AF = mybir.ActivationFunctionType

DIM = 1024
HEADS = 16
DHEAD = 64
B = 8
N = 256          # query tokens (x)
M = 256          # memory tokens (h)
T = M + N        # 512 keys
INNER = HEADS * DHEAD
SCALE = DHEAD ** -0.5
NEG = -30000.0   # fp16-representable; *0.125 still underflows exp
SW = 767         # BDs scratch width (relative offsets s = 1..767)
VAL0 = 255       # scratch col of first valid offset (s = 256)
NVALID = 257     # valid offsets s in [256, 512]
NV2 = 258        # band write width (one NEG pad col keeps mask intact)
WIN = 384        # per-query-block live key window (3 of 4 key tiles)
NBUF = 32        # BDs scratch buffers (one per iteration: no reuse, no WAR)
NIT = 32         # attention iterations (16 heads x 2 query blocks)
LAG = 4          # software-pipeline lookahead in the attention loop


def build_kernel():
    nc = bacc.Bacc("TRN2", target_bir_lowering=False, debug=False)

    x_d = nc.dram_tensor("x", [N, DIM], F32, kind="ExternalInput")
    h_d = nc.dram_tensor("h", [M, DIM], F32, kind="ExternalInput")
    wqkv_d = nc.dram_tensor("Wqkv", [DIM, 3 * INNER], F32, kind="ExternalInput")
    wkr_d = nc.dram_tensor("Wkr", [DIM, INNER], F32, kind="ExternalInput")
    r_d = nc.dram_tensor("R", [2 * T, DIM], F32, kind="ExternalInput")
    uu_d = nc.dram_tensor("uu", [128, 1], F32, kind="ExternalInput")
    vv_d = nc.dram_tensor("vv", [128, 1], F32, kind="ExternalInput")
    wout_d = nc.dram_tensor("Wout", [INNER, DIM], F32, kind="ExternalInput")
    out_d = nc.dram_tensor("out", [N, DIM], F32, kind="ExternalOutput")
    bds_d = nc.dram_tensor("bds_scratch", [NBUF, 128, SW], F16)
    junk_d = nc.dram_tensor("warm_junk", [128, 512], F16)

    with tile.TileContext(nc) as tc, ExitStack() as ctx:
        _body(ctx, tc, x_d, h_d, wqkv_d, wkr_d, r_d, uu_d, vv_d, wout_d,
              out_d, bds_d, junk_d)

    nc.compile()
    return nc


def _body(ctx, tc, x_d, h_d, wqkv_d, wkr_d, r_d, uu_d, vv_d, wout_d, out_d,
          bds_d, junk_d):
    nc = tc.nc

    const = ctx.enter_context(tc.tile_pool(name="const", bufs=1))
    persist = ctx.enter_context(tc.tile_pool(name="persist", bufs=1))
    ldpool = ctx.enter_context(tc.tile_pool(name="ld", bufs=4))
    work = ctx.enter_context(tc.tile_pool(name="work", bufs=4))
    ps_big = ctx.enter_context(tc.tile_pool(name="ps_big", bufs=4, space="PSUM"))
    ps_sml = ctx.enter_context(tc.tile_pool(name="ps_sml", bufs=2, space="PSUM"))
    ps_pav = ctx.enter_context(tc.tile_pool(name="ps_pav", bufs=2, space="PSUM"))

    # ---------------- PE warm-up (primes the HAM clock gate) ----------------
    junk = const.tile([128, 512], F16, tag="junk", name="junk")
    nc.vector.memset(junk, 1.0)
    pwarm = ps_big.tile([128, 512], F32, tag="big", name="ps_warm")
    for wi in range(16):
        nc.tensor.matmul(pwarm, junk[:, 0:128], junk,
                         start=(wi == 0), stop=(wi == 15))
    junk2 = const.tile([128, 512], F16, tag="junk2", name="junk2")
    nc.vector.tensor_copy(junk2, pwarm)
    nc.sync.dma_start(out=junk_d[:, :], in_=junk2)

    # ---------------- constants ----------------
    ident_h = const.tile([128, 128], F16, tag="identh", name="ident_h")
    make_identity(nc, ident_h)

    uu = const.tile([128, 1], F32, tag="uu", name="uu_sb")
    vv = const.tile([128, 1], F32, tag="vv", name="vv_sb")
    nc.sync.dma_start(out=uu, in_=uu_d[:, :])
    nc.sync.dma_start(out=vv, in_=vv_d[:, :])

    # ---------------- loads (single gpsimd SWDGE queue, ordered) ----------
    # cat token order: [h (0:256) | x (256:512)]; casts f32 -> f16 in flight.
    cat16 = []
    for tt in range(4):
        t_ = ldpool.tile([128, DIM], F16, tag="xh", name=f"cat16_{tt}")
        src = h_d if tt < 2 else x_d
        nc.gpsimd.dma_start(out=t_, in_=src[(tt % 2) * 128:(tt % 2) * 128 + 128, :])
        cat16.append(t_)

    # R rows needed: offsets s=256..511 -> rows 768..1023; s=512 -> row 0
    r16 = []
    for rt in range(2):
        t_ = ldpool.tile([128, DIM], F16, tag="rn", name=f"r16_{rt}", bufs=2)
        nc.gpsimd.dma_start(out=t_, in_=r_d[768 + rt * 128:768 + (rt + 1) * 128, :])
        r16.append(t_)
    # R row 0 (offset s=512), loaded directly transposed: partition p of
    # column dt holds R[0, dt*128 + p]
    r0T = const.tile([128, 8], F16, tag="r0T", name="r0T")
    nc.gpsimd.dma_start(out=r0T,
                        in_=bass.AP(r_d[0].tensor, 0, [[1, 128], [128, 8]]))

    wq16 = [persist.tile([128, INNER], F16, tag=f"wq16_{dt}", name=f"wq16_{dt}")
            for dt in range(8)]
    for dt in range(8):
        nc.gpsimd.dma_start(out=wq16[dt],
                            in_=wqkv_d[dt * 128:(dt + 1) * 128, 0:INNER])
    wkr16 = [persist.tile([128, INNER], F16, tag=f"wkr16_{dt}", name=f"wkr16_{dt}")
             for dt in range(8)]
    for dt in range(8):
        nc.gpsimd.dma_start(out=wkr16[dt], in_=wkr_d[dt * 128:(dt + 1) * 128, :])
    wk16 = [persist.tile([128, INNER], F16, tag=f"wk16_{dt}", name=f"wk16_{dt}")
            for dt in range(8)]
    for dt in range(8):
        nc.gpsimd.dma_start(out=wk16[dt],
                            in_=wqkv_d[dt * 128:(dt + 1) * 128, INNER:2 * INNER])
    wv16 = [persist.tile([128, INNER], F16, tag=f"wv16_{dt}", name=f"wv16_{dt}")
            for dt in range(8)]
    for dt in range(8):
        nc.gpsimd.dma_start(out=wv16[dt],
                            in_=wqkv_d[dt * 128:(dt + 1) * 128,
                                       2 * INNER:3 * INNER])
    wo16 = [persist.tile([128, DIM], F16, tag=f"wo16_{dt}", name=f"wo16_{dt}")
            for dt in range(8)]
    for dt in range(8):
        nc.gpsimd.dma_start(out=wo16[dt], in_=wout_d[dt * 128:(dt + 1) * 128, :])

    # ---------------- transposes of cat and R ----------------
    catT = [persist.tile([128, T], F16, tag=f"catT{dt}", name=f"catT{dt}")
            for dt in range(8)]
    for tt in range(4):
        for dt in range(8):
            pool = ps_sml if dt % 2 == 0 else ps_pav
            tp = pool.tile([128, 128], F16, tag="tp" if pool is ps_sml else "pav", name=f"tp_cat{tt}_{dt}")
            nc.tensor.transpose(tp, cat16[tt][:, dt * 128:(dt + 1) * 128],
                                ident_h)
            nc.vector.tensor_copy(catT[dt][:, tt * 128:(tt + 1) * 128], tp)

    rsubT = [persist.tile([128, NV2], F16, tag=f"rsubT{dt}", name=f"rsubT{dt}")
             for dt in range(8)]
    for rt in range(2):
        for dt in range(8):
            pool = ps_sml if dt % 2 == 0 else ps_pav
            tp = pool.tile([128, 128], F16, tag="tp" if pool is ps_sml else "pav", name=f"tp_r{rt}_{dt}")
            nc.tensor.transpose(tp, r16[rt][:, dt * 128:(dt + 1) * 128],
                                ident_h)
            nc.scalar.copy(rsubT[dt][:, rt * 128:(rt + 1) * 128], tp)
    for dt in range(8):
        nc.vector.tensor_copy(rsubT[dt][:, 256:257], r0T[:, dt:dt + 1])
        nc.vector.memset(rsubT[dt][:, 257:258], 0.0)

    # ---------------- q projection (dt-outer in ft pairs) ----------------
    # tracks the Wq load stream: the dt loop is outermost so each weight tile
    # is consumed as it arrives instead of waiting for the full matrix.
    quT = [persist.tile([128, N], F16, tag=f"quT{ft}", name=f"quT{ft}")
           for ft in range(8)]
    qvT = [persist.tile([128, N], F16, tag=f"qvT{ft}", name=f"qvT{ft}")
           for ft in range(8)]
    for g in range(4):
        qpool, qtag = (ps_big, "big") if g % 2 == 0 else (ps_pav, "pav")
        pq = [qpool.tile([128, N], F32, tag=qtag, name=f"ps_q{g}_{j}")
              for j in range(2)]
        for dt in range(8):
            for j in range(2):
                ft = 2 * g + j
                nc.tensor.matmul(pq[j], wq16[dt][:, ft * 128:(ft + 1) * 128],
                                 catT[dt][:, M:T], start=(dt == 0),
                                 stop=(dt == 7))
        for j in range(2):
            ft = 2 * g + j
            nc.vector.tensor_scalar_add(quT[ft], pq[j], uu)
            nc.vector.tensor_scalar_add(qvT[ft], pq[j], vv)

    # ---------------- RWs projection (dt-outer in ft pairs) ----------------
    rwsT = [persist.tile([128, NV2], F16, tag=f"rwsT{ft}", name=f"rwsT{ft}")
            for ft in range(8)]
    for g in range(4):
        rpool, rtag = (ps_big, "big") if g % 2 == 0 else (ps_pav, "pav")
        pr = [rpool.tile([128, NV2], F32, tag=rtag, name=f"ps_rw{g}_{j}")
              for j in range(2)]
        for dt in range(8):
            for j in range(2):
                ft = 2 * g + j
                nc.tensor.matmul(pr[j], wkr16[dt][:, ft * 128:(ft + 1) * 128],
                                 rsubT[dt], start=(dt == 0), stop=(dt == 7))
        for j in range(2):
            nc.scalar.copy(rwsT[2 * g + j], pr[j])

    # ---------------- k projection (dt-outer in ft pairs) ----------------
    kT = [persist.tile([128, T], F16, tag=f"kT{ft}", name=f"kT{ft}")
          for ft in range(8)]
    for g in range(4):
        kpool, ktag = (ps_big, "big") if g % 2 == 0 else (ps_pav, "pav")
        pk = [kpool.tile([128, T], F32, tag=ktag, name=f"ps_k{g}_{j}")
              for j in range(2)]
        for dt in range(8):
            for j in range(2):
                ft = 2 * g + j
                nc.tensor.matmul(pk[j], wk16[dt][:, ft * 128:(ft + 1) * 128],
                                 catT[dt], start=(dt == 0), stop=(dt == 7))
        for j in range(2):
            nc.vector.tensor_copy(kT[2 * g + j], pk[j])

    # ---------------- fused BD + attention pipeline ----------------
    # Stages, offset in pipeline steps (1 step = 1 iteration = head x qb):
    #   bd(s):    BD matmul + band staging into a 4-iteration batch tile;
    #             one [128, 2048] write DMA per 4 iterations covers the full
    #             shear-read window incl. NEG mask columns (no scratch
    #             pre-init, few ring DMAs - the HWDGE ring is ~1/1.1us).
    #   a(s-4):   A matmul; one batched band read DMA per 4 iterations.
    #   mid(s-6): dots = pa + band (drains pa psum), exp + rowsum.
    #   back(s-8): 1/S normalize, 3 transposes into the head's psum bank.
    # Value projection is interleaved once Wv has streamed in.
    val = [persist.tile([128, INNER], F16, tag=f"val{tt}", name=f"val{tt}")
           for tt in range(4)]
    attnT = {}   # (hh, jt) -> tile
    tpt = {}     # hh -> packed psum tile
    attn_outT = [persist.tile([128, N], F16, tag=f"aoT{ft}", name=f"aoT{ft}")
                 for ft in range(8)]

    pa_t = [None] * NIT
    band_t = [None] * NIT
    expt_t = [None] * NIT
    bsb_t = [None] * (NIT // 4)
    w_insts = [None] * (NIT // 4)
    ncopy = 0

    def bd(it):
        hh, qb = it // 2, it % 2
        ft, ro = hh // 2, (hh % 2) * 64
        qsl = slice(qb * 128, (qb + 1) * 128)
        b = it // 4
        if it % 4 == 0:
            bsb_t[b] = work.tile([128, 2048], F16, tag="bsb",
                                 name=f"bsb{b}", bufs=3)
        bsb = bsb_t[b]
        pb = ps_big.tile([128, NV2], F32, tag="big", name=f"ps_b{it}")
        nc.tensor.matmul(pb, qvT[ft][ro:ro + 64, qsl],
                         rwsT[ft][ro:ro + 64, :], start=True, stop=True)
        o = (it % 4) * 512
        meng = nc.vector if it < 16 else nc.gpsimd
        meng.memset(bsb[:, o:o + 127], NEG)
        nc.vector.tensor_copy(bsb[:, o + 127:o + 127 + NVALID],
                              pb[:, 0:NVALID])
        meng.memset(bsb[:, o + 384:o + 512], NEG)
        if it % 4 == 3:
            # scratch bufs [4b, 4b+4), cols [128, 640)
            dst = bass.AP(bds_d[0].tensor, 4 * b * 128 * SW + 128,
                          [[SW, 128], [128 * SW, 4], [1, 512]])
            eng = nc.sync if b % 2 == 0 else nc.scalar
            w_insts[b] = eng.dma_start(out=dst, in_=bsb)

    def front(it):
        hh, qb = it // 2, it % 2
        ft, ro = hh // 2, (hh % 2) * 64
        qsl = slice(qb * 128, (qb + 1) * 128)
        if it % 4 == 0:
            b = it // 4
            band4 = work.tile([128, 4 * WIN], F16, tag="band",
                              name=f"band{b}", bufs=4)
            src = bass.AP(bds_d[0].tensor, 4 * b * 128 * SW + VAL0,
                          [[SW - 1, 128], [128 * SW, 4], [1, WIN]])
            eng = nc.scalar if b % 2 == 0 else nc.sync
            r_inst = eng.dma_start(out=band4, in_=src)
            add_dep_helper(r_inst.ins, w_insts[b].ins, sync=True,
                           reason="band RAW on scratch")
            for j in range(4):
                band_t[4 * b + j] = band4[:, j * WIN:(j + 1) * WIN]
        pa = ps_big.tile([128, WIN], F32, tag="big", name=f"ps_a{it}")
        nc.tensor.matmul(pa, quT[ft][ro:ro + 64, qsl],
                         kT[ft][ro:ro + 64, qb * 128:qb * 128 + WIN],
                         start=True, stop=True)
        pa_t[it] = pa

    def mid(it):
        dots = work.tile([128, WIN], F16, tag="dots", name=f"dots{it}", bufs=5)
        nc.vector.tensor_add(dots, pa_t[it], band_t[it])
        expt = work.tile([128, WIN], F16, tag="expt", name=f"expt{it}", bufs=5)
        ssum = work.tile([128, 1], F32, tag="ssum", name=f"ssum{it}", bufs=6)
        nc.scalar.activation(expt, dots, AF.Exp, bias=0.0, scale=SCALE,
                             accum_out=ssum)
        expt_t[it] = (expt, ssum)

    def back(it):
        nonlocal ncopy
        hh, qb = it // 2, it % 2
        expt, ssum = expt_t[it]
        rcp = work.tile([128, 1], F32, tag="rcp", name=f"rcp{it}", bufs=6)
        nc.vector.reciprocal(rcp, ssum)
        expn = work.tile([128, WIN], F16, tag="expn", name=f"expn{it}",
                         bufs=6)
        neng = nc.vector if it < 16 else nc.gpsimd
        neng.tensor_scalar_mul(expn, expt, rcp)
        # per-head packed psum layout (one bank):
        #   [jt0: 0:128 | jt1: 128:384 | jt2: 384:640 | jt3: 640:768]
        # within jt1/jt2, qb0 occupies the first 128 cols, qb1 the second.
        if hh not in tpt:
            tpt[hh] = ps_sml.tile([128, 1024], F16, tag="tp",
                                  name=f"tpt{hh}")
        offs = ([0, 128, 384] if qb == 0 else [256, 512, 640])
        for w in range(3):
            nc.tensor.transpose(tpt[hh][:, offs[w]:offs[w] + 128],
                                expn[:, w * 128:(w + 1) * 128], ident_h)
        if qb == 1:
            # drain the head's 4 key-major attention tiles
            for jt, lo, width in ((0, 0, 128), (1, 128, 256),
                                  (2, 384, 256), (3, 640, 128)):
                at = persist.tile([128, width], F16, tag=f"at{hh}_{jt}",
                                  name=f"attnT{hh}_{jt}")
                src = tpt[hh][:, lo:lo + width]
                if ncopy % 2 == 0:
                    nc.vector.tensor_copy(at, src)
                else:
                    nc.scalar.copy(at, src)
                ncopy += 1
                attnT[(hh, jt)] = at

    def vproj(tt):
        # value projection for token tile tt (needs full Wv)
        vpool, vtag = (ps_big, "big") if tt % 2 == 0 else (ps_pav, "pav")
        pv = [vpool.tile([128, 512], F32, tag=vtag, name=f"ps_v{tt}_{nh}")
              for nh in range(2)]
        for dt in range(8):
            lhs = catT[dt][:, tt * 128:(tt + 1) * 128]
            for nh in range(2):
                nc.tensor.matmul(pv[nh], lhs,
                                 wv16[dt][:, nh * 512:(nh + 1) * 512],
                                 start=(dt == 0), stop=(dt == 7))
        for nh in range(2):
            nc.vector.tensor_copy(val[tt][:, nh * 512:(nh + 1) * 512], pv[nh])

    vp_after = {13: 0, 18: 1, 23: 2, 28: 3}
    for s in range(NIT + 8):
        if s < NIT:
            bd(s)
        if 0 <= s - 4 < NIT:
            front(s - 4)
        if 0 <= s - 6 < NIT:
            mid(s - 6)
        if 0 <= s - 8 < NIT:
            back(s - 8)
        if s in vp_after:
            vproj(vp_after[s])

    # ---------------- AV + output head ----------------
    for hh in range(HEADS):
        ft, ro = hh // 2, (hh % 2) * 64
        pav = ps_pav.tile([64, N], F32, tag="pav", name=f"ps_av{hh}")
        # left half (queries 0:128): keys jt 0,1,2
        nc.tensor.matmul(pav[:, 0:128], val[0][:, hh * 64:hh * 64 + 64],
                         attnT[(hh, 0)], start=True, stop=False,
                         skip_group_check=True)
        nc.tensor.matmul(pav[:, 0:128], val[1][:, hh * 64:hh * 64 + 64],
                         attnT[(hh, 1)][:, 0:128], start=False, stop=False,
                         skip_group_check=True)
        nc.tensor.matmul(pav[:, 0:128], val[2][:, hh * 64:hh * 64 + 64],
                         attnT[(hh, 2)][:, 0:128], start=False, stop=True,
                         skip_group_check=True)
        # right half (queries 128:256): keys jt 1,2,3
        nc.tensor.matmul(pav[:, 128:256], val[1][:, hh * 64:hh * 64 + 64],
                         attnT[(hh, 1)][:, 128:256], start=True, stop=False,
                         skip_group_check=True)
        nc.tensor.matmul(pav[:, 128:256], val[2][:, hh * 64:hh * 64 + 64],
                         attnT[(hh, 2)][:, 128:256], start=False, stop=False,
                         skip_group_check=True)
        nc.tensor.matmul(pav[:, 128:256], val[3][:, hh * 64:hh * 64 + 64],
                         attnT[(hh, 3)], start=False, stop=True,
                         skip_group_check=True)
        if hh % 2 == 0:
            nc.vector.tensor_copy(attn_outT[ft][ro:ro + 64, :], pav)
        else:
            nc.scalar.copy(attn_outT[ft][ro:ro + 64, :], pav)

    # ---------------- output projection ----------------
    for tt in range(2):
        pp = [ps_big.tile([128, 512], F32, tag="big", name=f"ps_o{tt}_{nh}")
              for nh in range(2)]
        for itile in range(8):
            lhs = attn_outT[itile][:, tt * 128:(tt + 1) * 128]
            for nh in range(2):
                nc.tensor.matmul(pp[nh], lhs,
                                 wo16[itile][:, nh * 512:(nh + 1) * 512],
                                 start=(itile == 0), stop=(itile == 7))
        osb = work.tile([128, DIM], F32, tag="osb", name=f"osb{tt}", bufs=2)
        nc.vector.tensor_copy(osb[:, 0:512], pp[0])
        nc.scalar.copy(osb[:, 512:1024], pp[1])
        nc.sync.dma_start(out=out_d[tt * 128:(tt + 1) * 128, :], in_=osb)


_NC_CACHE = {}


def _get_nc():
    if "nc" not in _NC_CACHE:
        _NC_CACHE["nc"] = build_kernel()
    return _NC_CACHE["nc"]


def _run(inputs, trace=False):
    x = np.ascontiguousarray(np.asarray(inputs["x"], dtype=np.float32))
    h = np.ascontiguousarray(np.asarray(inputs["h"], dtype=np.float32))
    wqkv = np.ascontiguousarray(np.asarray(inputs["Wqkv"], dtype=np.float32))
    wkr = np.ascontiguousarray(np.asarray(inputs["Wkr"], dtype=np.float32))
    r = np.ascontiguousarray(np.asarray(inputs["R"], dtype=np.float32))
    u = np.asarray(inputs["u"], dtype=np.float32)
    v = np.asarray(inputs["v"], dtype=np.float32)
    wout = np.ascontiguousarray(np.asarray(inputs["Wout"], dtype=np.float32))
    uu = np.ascontiguousarray(np.tile(u, 2).reshape(128, 1))
    vv = np.ascontiguousarray(np.tile(v, 2).reshape(128, 1))

    nc = _get_nc()
    in_maps = [
        {"x": x[b], "h": h[b], "Wqkv": wqkv, "Wkr": wkr, "R": r,
         "uu": uu, "vv": vv, "Wout": wout}
        for b in range(B)
    ]
    res = bass_utils.run_bass_kernel_spmd(
        nc, in_maps, core_ids=list(range(B)), trace=trace)
    out = np.stack([res.results[b]["out"] for b in range(B)])
    return out.astype(np.float32), res


def kernel(**inputs):
    out, _ = _run(inputs, trace=False)
    return out


# revision 27
# speedup vs baseline: 1.4930x; 1.4930x over previous
# Transformer-XL style relative-position attention on 8 Trainium2 NeuronCores.
#
# Contract: kernel(**inputs) takes the FULL unsharded inputs and returns the
# FULL [8, 256, 1024] output. Internally shards data-parallel over batch:
# core b computes batch element b. No collectives needed.
#
# Math (per batch element):
#   cat = [h; x]                            [512, 1024]
#   q,k,v = split(cat @ Wqkv)               heads=16, dhead=64
#   RW    = R @ Wkr                         [1024, 1024] (relative pos keys)
#   dots  = (q+u) @ k^T + rel_shift((q+v) @ RW_h^T)
#   out   = softmax(dots*8^-1 + causal/mem band mask) @ v @ Wout
#
# Key design points (v2 — restructured for load/compute overlap):
#  * Only 257 rows of RW are ever needed (offsets s in [256, 512]).
#  * rel_shift via DRAM scratch shear: write the [128, 258] BD band to a
#    [128, 767] scratch row and read back with AP [[766, 128], [1, 384]],
#    which realizes band[i, j] = BD[i, j - i + c]. Scratch pre-filled with
#    NEG outside the band so the read returns band+mask in one tensor.
#  * The whole schedule is arranged around the HBM load stream (24 MB of
#    f32 weights at ~370 GB/s ~= 62 us): projections run dt-outer in
#    ft-pairs so they track the arrival of each weight tile; the 32 BD
#    matmuls + band writes flood the PE/sync queues while Wk streams in;
#    the A/exp/transpose pipeline runs while Wv/Wout stream; AV + out
#    projection form the tail.
#  * Band writes and reads are split across the two HWDGE rings (sync and
#    scalar) to halve per-ring serialization.
#  * Attention inner loop is software-pipelined with explicit lookahead so
#    no engine FIFO head-blocks on the DRAM shear round trip.
#  * Normalization 1/S rides the mandatory exp transpose via diag(1/S).
#  * All matmul operands fp16; psum f32 except the exp transposes (f16).

import numpy as np

import concourse.bass as bass
import concourse.mybir as mybir
import concourse.tile as tile
from concourse import bacc, bass_utils
from concourse.masks import make_identity
from concourse.tile import add_dep_helper
from contextlib import ExitStack

F32 = mybir.dt.float32
F16 = mybir.dt.float16
AF = mybir.ActivationFunctionType

DIM = 1024
HEADS = 16
DHEAD = 64
B = 8
N = 256          # query tokens (x)
M = 256          # memory tokens (h)
T = M + N        # 512 keys
INNER = HEADS * DHEAD
SCALE = DHEAD ** -0.5
NEG = -30000.0   # fp16-representable; *0.125 still underflows exp
SW = 767         # BDs scratch width (relative offsets s = 1..767)
VAL0 = 255       # scratch col of first valid offset (s = 256)
NVALID = 257     # valid offsets s in [256, 512]
NV2 = 258        # band write width (one NEG pad col keeps mask intact)
WIN = 384        # per-query-block live key window (3 of 4 key tiles)
NBUF = 32        # BDs scratch buffers (one per iteration: no reuse, no WAR)
NIT = 32         # attention iterations (16 heads x 2 query blocks)
LAG = 4          # software-pipeline lookahead in the attention loop


def build_kernel():
    nc = bacc.Bacc("TRN2", target_bir_lowering=False, debug=False)

    x_d = nc.dram_tensor("x", [N, DIM], F32, kind="ExternalInput")
    h_d = nc.dram_tensor("h", [M, DIM], F32, kind="ExternalInput")
    wqkv_d = nc.dram_tensor("Wqkv", [DIM, 3 * INNER], F32, kind="ExternalInput")
    wkr_d = nc.dram_tensor("Wkr", [DIM, INNER], F32, kind="ExternalInput")
    r_d = nc.dram_tensor("R", [2 * T, DIM], F32, kind="ExternalInput")
    uu_d = nc.dram_tensor("uu", [128, 1], F32, kind="ExternalInput")
    vv_d = nc.dram_tensor("vv", [128, 1], F32, kind="ExternalInput")
    wout_d = nc.dram_tensor("Wout", [INNER, DIM], F32, kind="ExternalInput")
    out_d = nc.dram_tensor("out", [N, DIM], F32, kind="ExternalOutput")
    bds_d = nc.dram_tensor("bds_scratch", [NBUF, 128, SW], F16)
    junk_d = nc.dram_tensor("warm_junk", [128, 512], F16)

    with tile.TileContext(nc) as tc, ExitStack() as ctx:
        _body(ctx, tc, x_d, h_d, wqkv_d, wkr_d, r_d, uu_d, vv_d, wout_d,
              out_d, bds_d, junk_d)

    nc.compile()
    return nc


def _body(ctx, tc, x_d, h_d, wqkv_d, wkr_d, r_d, uu_d, vv_d, wout_d, out_d,
          bds_d, junk_d):
    nc = tc.nc

    const = ctx.enter_context(tc.tile_pool(name="const", bufs=1))
    persist = ctx.enter_context(tc.tile_pool(name="persist", bufs=1))
    ldpool = ctx.enter_context(tc.tile_pool(name="ld", bufs=4))
    work = ctx.enter_context(tc.tile_pool(name="work", bufs=4))
    ps_big = ctx.enter_context(tc.tile_pool(name="ps_big", bufs=4, space="PSUM"))
    ps_sml = ctx.enter_context(tc.tile_pool(name="ps_sml", bufs=2, space="PSUM"))
    ps_pav = ctx.enter_context(tc.tile_pool(name="ps_pav", bufs=2, space="PSUM"))

    # ---------------- PE warm-up (primes the HAM clock gate) ----------------
    junk = const.tile([128, 512], F16, tag="junk", name="junk")
    nc.vector.memset(junk, 1.0)
    pwarm = ps_big.tile([128, 512], F32, tag="big", name="ps_warm")
    for wi in range(16):
        nc.tensor.matmul(pwarm, junk[:, 0:128], junk,
                         start=(wi == 0), stop=(wi == 15))
    junk2 = const.tile([128, 512], F16, tag="junk2", name="junk2")
    nc.vector.tensor_copy(junk2, pwarm)
    nc.sync.dma_start(out=junk_d[:, :], in_=junk2)

    # ---------------- constants ----------------
    ident_h = const.tile([128, 128], F16, tag="identh", name="ident_h")
    make_identity(nc, ident_h)

    uu = const.tile([128, 1], F32, tag="uu", name="uu_sb")
    vv = const.tile([128, 1], F32, tag="vv", name="vv_sb")
    nc.sync.dma_start(out=uu, in_=uu_d[:, :])
    nc.sync.dma_start(out=vv, in_=vv_d[:, :])

    # ---------------- loads (single gpsimd SWDGE queue, ordered) ----------
    # cat token order: [h (0:256) | x (256:512)]; casts f32 -> f16 in flight.
    cat16 = []
    for tt in range(4):
        t_ = ldpool.tile([128, DIM], F16, tag="xh", name=f"cat16_{tt}")
        src = h_d if tt < 2 else x_d
        nc.gpsimd.dma_start(out=t_, in_=src[(tt % 2) * 128:(tt % 2) * 128 + 128, :])
        cat16.append(t_)

    # R rows needed: offsets s=256..511 -> rows 768..1023; s=512 -> row 0
    r16 = []
    for rt in range(2):
        t_ = ldpool.tile([128, DIM], F16, tag="rn", name=f"r16_{rt}", bufs=2)
        nc.gpsimd.dma_start(out=t_, in_=r_d[768 + rt * 128:768 + (rt + 1) * 128, :])
        r16.append(t_)
    # R row 0 (offset s=512), loaded directly transposed: partition p of
    # column dt holds R[0, dt*128 + p]
    r0T = const.tile([128, 8], F16, tag="r0T", name="r0T")
    nc.gpsimd.dma_start(out=r0T,
                        in_=bass.AP(r_d[0].tensor, 0, [[1, 128], [128, 8]]))

    wq16 = [persist.tile([128, INNER], F16, tag=f"wq16_{dt}", name=f"wq16_{dt}")
            for dt in range(8)]
    for dt in range(8):
        nc.gpsimd.dma_start(out=wq16[dt],
                            in_=wqkv_d[dt * 128:(dt + 1) * 128, 0:INNER])
    wkr16 = [persist.tile([128, INNER], F16, tag=f"wkr16_{dt}", name=f"wkr16_{dt}")
             for dt in range(8)]
    for dt in range(8):
        nc.gpsimd.dma_start(out=wkr16[dt], in_=wkr_d[dt * 128:(dt + 1) * 128, :])
    wk16 = [persist.tile([128, INNER], F16, tag=f"wk16_{dt}", name=f"wk16_{dt}")
            for dt in range(8)]
    for dt in range(8):
        nc.gpsimd.dma_start(out=wk16[dt],
                            in_=wqkv_d[dt * 128:(dt + 1) * 128, INNER:2 * INNER])
    wv16 = [persist.tile([128, INNER], F16, tag=f"wv16_{dt}", name=f"wv16_{dt}")
            for dt in range(8)]
    for dt in range(8):
        nc.gpsimd.dma_start(out=wv16[dt],
                            in_=wqkv_d[dt * 128:(dt + 1) * 128,
                                       2 * INNER:3 * INNER])
    wo16 = [persist.tile([128, DIM], F16, tag=f"wo16_{dt}", name=f"wo16_{dt}")
            for dt in range(8)]
    for dt in range(8):
        nc.gpsimd.dma_start(out=wo16[dt], in_=wout_d[dt * 128:(dt + 1) * 128, :])

    # ---------------- transposes of cat and R ----------------
    catT = [persist.tile([128, T], F16, tag=f"catT{dt}", name=f"catT{dt}")
            for dt in range(8)]
    for tt in range(4):
        for dt in range(8):
            pool = ps_sml if dt % 2 == 0 else ps_pav
            tp = pool.tile([128, 128], F16, tag="tp" if pool is ps_sml else "pav", name=f"tp_cat{tt}_{dt}")
            nc.tensor.transpose(tp, cat16[tt][:, dt * 128:(dt + 1) * 128],
                                ident_h)
            nc.vector.tensor_copy(catT[dt][:, tt * 128:(tt + 1) * 128], tp)

    rsubT = [persist.tile([128, NV2], F16, tag=f"rsubT{dt}", name=f"rsubT{dt}")
             for dt in range(8)]
    for rt in range(2):
        for dt in range(8):
            pool = ps_sml if dt % 2 == 0 else ps_pav
            tp = pool.tile([128, 128], F16, tag="tp" if pool is ps_sml else "pav", name=f"tp_r{rt}_{dt}")
            nc.tensor.transpose(tp, r16[rt][:, dt * 128:(dt + 1) * 128],
                                ident_h)
            nc.scalar.copy(rsubT[dt][:, rt * 128:(rt + 1) * 128], tp)
    for dt in range(8):
        nc.vector.tensor_copy(rsubT[dt][:, 256:257], r0T[:, dt:dt + 1])
        nc.vector.memset(rsubT[dt][:, 257:258], 0.0)

    # ---------------- q projection (dt-outer in ft pairs) ----------------
    # tracks the Wq load stream: the dt loop is outermost so each weight tile
    # is consumed as it arrives instead of waiting for the full matrix.
    quT = [persist.tile([128, N], F16, tag=f"quT{ft}", name=f"quT{ft}")
           for ft in range(8)]
    qvT = [persist.tile([128, N], F16, tag=f"qvT{ft}", name=f"qvT{ft}")
           for ft in range(8)]
    for g in range(4):
        qpool, qtag = (ps_big, "big") if g % 2 == 0 else (ps_pav, "pav")
        pq = [qpool.tile([128, N], F32, tag=qtag, name=f"ps_q{g}_{j}")
              for j in range(2)]
        for dt in range(8):
            for j in range(2):
                ft = 2 * g + j
                nc.tensor.matmul(pq[j], wq16[dt][:, ft * 128:(ft + 1) * 128],
                                 catT[dt][:, M:T], start=(dt == 0),
                                 stop=(dt == 7))
        for j in range(2):
            ft = 2 * g + j
            nc.vector.tensor_scalar_add(quT[ft], pq[j], uu)
            nc.vector.tensor_scalar_add(qvT[ft], pq[j], vv)

    # ---------------- RWs projection (dt-outer in ft pairs) ----------------
    rwsT = [persist.tile([128, NV2], F16, tag=f"rwsT{ft}", name=f"rwsT{ft}")
            for ft in range(8)]
    for g in range(4):
        rpool, rtag = (ps_big, "big") if g % 2 == 0 else (ps_pav, "pav")
        pr = [rpool.tile([128, NV2], F32, tag=rtag, name=f"ps_rw{g}_{j}")
              for j in range(2)]
        for dt in range(8):
            for j in range(2):
                ft = 2 * g + j
                nc.tensor.matmul(pr[j], wkr16[dt][:, ft * 128:(ft + 1) * 128],
                                 rsubT[dt], start=(dt == 0), stop=(dt == 7))
        for j in range(2):
            nc.scalar.copy(rwsT[2 * g + j], pr[j])

    # ---------------- k projection (dt-outer in ft pairs) ----------------
    kT = [persist.tile([128, T], F16, tag=f"kT{ft}", name=f"kT{ft}")
          for ft in range(8)]
    for g in range(4):
        kpool, ktag = (ps_big, "big") if g % 2 == 0 else (ps_pav, "pav")
        pk = [kpool.tile([128, T], F32, tag=ktag, name=f"ps_k{g}_{j}")
              for j in range(2)]
        for dt in range(8):
            for j in range(2):
                ft = 2 * g + j
                nc.tensor.matmul(pk[j], wk16[dt][:, ft * 128:(ft + 1) * 128],
                                 catT[dt], start=(dt == 0), stop=(dt == 7))
        for j in range(2):
            nc.vector.tensor_copy(kT[2 * g + j], pk[j])

    # ---------------- fused BD + attention pipeline ----------------
    # Stages, offset in pipeline steps (1 step = 1 iteration = head x qb):
    #   bd(s):    BD matmul + band staging into a 4-iteration batch tile;
    #             one [128, 2048] write DMA per 4 iterations covers the full
    #             shear-read window incl. NEG mask columns (no scratch
    #             pre-init, few ring DMAs - the HWDGE ring is ~1/1.1us).
    #   a(s-4):   A matmul; one batched band read DMA per 4 iterations.
    #   mid(s-6): dots = pa + band (drains pa psum), exp + rowsum.
    #   back(s-8): 1/S normalize, 3 transposes into the head's psum bank.
    # Value projection is interleaved once Wv has streamed in.
    val = [persist.tile([128, INNER], F16, tag=f"val{tt}", name=f"val{tt}")
           for tt in range(4)]
    attnT = {}   # (hh, jt) -> tile
    tpt = {}     # hh -> packed psum tile
    attn_outT = [persist.tile([128, N], F16, tag=f"aoT{ft}", name=f"aoT{ft}")
                 for ft in range(8)]

    pa_t = [None] * NIT
    band_t = [None] * NIT
    expt_t = [None] * NIT
    bsb_t = [None] * (NIT // 4)
    w_insts = [None] * (NIT // 4)
    ncopy = 0

    def bd(it):
        hh, qb = it // 2, it % 2
        ft, ro = hh // 2, (hh % 2) * 64
        qsl = slice(qb * 128, (qb + 1) * 128)
        b = it // 4
        if it % 4 == 0:
            bsb_t[b] = work.tile([128, 2048], F16, tag="bsb",
                                 name=f"bsb{b}", bufs=3)
        bsb = bsb_t[b]
        pb = ps_big.tile([128, NV2], F32, tag="big", name=f"ps_b{it}")
        nc.tensor.matmul(pb, qvT[ft][ro:ro + 64, qsl],
                         rwsT[ft][ro:ro + 64, :], start=True, stop=True)
        o = (it % 4) * 512
        nc.vector.memset(bsb[:, o:o + 127], NEG)
        nc.vector.tensor_copy(bsb[:, o + 127:o + 127 + NVALID],
                              pb[:, 0:NVALID])
        nc.vector.memset(bsb[:, o + 384:o + 512], NEG)
        if it % 4 == 3:
            # scratch bufs [4b, 4b+4), cols [128, 640)
            dst = bass.AP(bds_d[0].tensor, 4 * b * 128 * SW + 128,
                          [[SW, 128], [128 * SW, 4], [1, 512]])
            eng = nc.sync if b % 2 == 0 else nc.scalar
            w_insts[b] = eng.dma_start(out=dst, in_=bsb)

    def front(it):
        hh, qb = it // 2, it % 2
        ft, ro = hh // 2, (hh % 2) * 64
        qsl = slice(qb * 128, (qb + 1) * 128)
        if it % 4 == 0:
            b = it // 4
            band4 = work.tile([128, 4 * WIN], F16, tag="band",
                              name=f"band{b}", bufs=4)
            src = bass.AP(bds_d[0].tensor, 4 * b * 128 * SW + VAL0,
                          [[SW - 1, 128], [128 * SW, 4], [1, WIN]])
            eng = nc.scalar if b % 2 == 0 else nc.sync
            r_inst = eng.dma_start(out=band4, in_=src)
            add_dep_helper(r_inst.ins, w_insts[b].ins, sync=True,
                           reason="band RAW on scratch")
            for j in range(4):
                band_t[4 * b + j] = band4[:, j * WIN:(j + 1) * WIN]
        pa = ps_big.tile([128, WIN], F32, tag="big", name=f"ps_a{it}")
        nc.tensor.matmul(pa, quT[ft][ro:ro + 64, qsl],
                         kT[ft][ro:ro + 64, qb * 128:qb * 128 + WIN],
                         start=True, stop=True)
        pa_t[it] = pa

    def mid(it):
        dots = work.tile([128, WIN], F16, tag="dots", name=f"dots{it}", bufs=5)
        nc.vector.tensor_add(dots, pa_t[it], band_t[it])
        expt = work.tile([128, WIN], F16, tag="expt", name=f"expt{it}", bufs=5)
        ssum = work.tile([128, 1], F32, tag="ssum", name=f"ssum{it}", bufs=6)
        nc.scalar.activation(expt, dots, AF.Exp, bias=0.0, scale=SCALE,
                             accum_out=ssum)
        expt_t[it] = (expt, ssum)

    def back(it):
        nonlocal ncopy
        hh, qb = it // 2, it % 2
        expt, ssum = expt_t[it]
        rcp = work.tile([128, 1], F32, tag="rcp", name=f"rcp{it}", bufs=6)
        nc.vector.reciprocal(rcp, ssum)
        expn = work.tile([128, WIN], F16, tag="expn", name=f"expn{it}",
                         bufs=6)
        nc.vector.tensor_scalar_mul(expn, expt, rcp)
        # per-head packed psum layout (one bank):
        #   [jt0: 0:128 | jt1: 128:384 | jt2: 384:640 | jt3: 640:768]
        # within jt1/jt2, qb0 occupies the first 128 cols, qb1 the second.
        if hh not in tpt:
            tpt[hh] = ps_sml.tile([128, 1024], F16, tag="tp",
                                  name=f"tpt{hh}")
        offs = ([0, 128, 384] if qb == 0 else [256, 512, 640])
        for w in range(3):
            nc.tensor.transpose(tpt[hh][:, offs[w]:offs[w] + 128],
                                expn[:, w * 128:(w + 1) * 128], ident_h)
        if qb == 1:
            # drain the head's 4 key-major attention tiles
            for jt, lo, width in ((0, 0, 128), (1, 128, 256),
                                  (2, 384, 256), (3, 640, 128)):
                at = persist.tile([128, width], F16, tag=f"at{hh}_{jt}",
                                  name=f"attnT{hh}_{jt}")
                src = tpt[hh][:, lo:lo + width]
                if ncopy % 2 == 0:
                    nc.vector.tensor_copy(at, src)
                else:
                    nc.scalar.copy(at, src)
                ncopy += 1
                attnT[(hh, jt)] = at

    def vproj(tt):
        # value projection for token tile tt (needs full Wv)
        vpool, vtag = (ps_big, "big") if tt % 2 == 0 else (ps_pav, "pav")
        pv = [vpool.tile([128, 512], F32, tag=vtag, name=f"ps_v{tt}_{nh}")
              for nh in range(2)]
        for dt in range(8):
            lhs = catT[dt][:, tt * 128:(tt + 1) * 128]
            for nh in range(2):
                nc.tensor.matmul(pv[nh], lhs,
                                 wv16[dt][:, nh * 512:(nh + 1) * 512],
                                 start=(dt == 0), stop=(dt == 7))
        for nh in range(2):
            nc.vector.tensor_copy(val[tt][:, nh * 512:(nh + 1) * 512], pv[nh])

    vp_after = {13: 0, 18: 1, 23: 2, 28: 3}
    for s in range(NIT + 8):
        if s < NIT:
            bd(s)
        if 0 <= s - 4 < NIT:
            front(s - 4)
        if 0 <= s - 6 < NIT:
            mid(s - 6)
        if 0 <= s - 8 < NIT:
            back(s - 8)
        if s in vp_after:
            vproj(vp_after[s])

    # ---------------- AV + output head ----------------
    for hh in range(HEADS):
        ft, ro = hh // 2, (hh % 2) * 64
        pav = ps_pav.tile([64, N], F32, tag="pav", name=f"ps_av{hh}")
        # left half (queries 0:128): keys jt 0,1,2
        nc.tensor.matmul(pav[:, 0:128], val[0][:, hh * 64:hh * 64 + 64],
                         attnT[(hh, 0)], start=True, stop=False,
                         skip_group_check=True)
        nc.tensor.matmul(pav[:, 0:128], val[1][:, hh * 64:hh * 64 + 64],
                         attnT[(hh, 1)][:, 0:128], start=False, stop=False,
                         skip_group_check=True)
        nc.tensor.matmul(pav[:, 0:128], val[2][:, hh * 64:hh * 64 + 64],
                         attnT[(hh, 2)][:, 0:128], start=False, stop=True,
                         skip_group_check=True)
        # right half (queries 128:256): keys jt 1,2,3
        nc.tensor.matmul(pav[:, 128:256], val[1][:, hh * 64:hh * 64 + 64],
                         attnT[(hh, 1)][:, 128:256], start=True, stop=False,
                         skip_group_check=True)
        nc.tensor.matmul(pav[:, 128:256], val[2][:, hh * 64:hh * 64 + 64],
                         attnT[(hh, 2)][:, 128:256], start=False, stop=False,
                         skip_group_check=True)
        nc.tensor.matmul(pav[:, 128:256], val[3][:, hh * 64:hh * 64 + 64],
                         attnT[(hh, 3)], start=False, stop=True,
                         skip_group_check=True)
        if hh % 2 == 0:
            nc.vector.tensor_copy(attn_outT[ft][ro:ro + 64, :], pav)
        else:
            nc.scalar.copy(attn_outT[ft][ro:ro + 64, :], pav)

    # ---------------- output projection ----------------
    for tt in range(2):
        pp = [ps_big.tile([128, 512], F32, tag="big", name=f"ps_o{tt}_{nh}")
              for nh in range(2)]
        for itile in range(8):
            lhs = attn_outT[itile][:, tt * 128:(tt + 1) * 128]
            for nh in range(2):
                nc.tensor.matmul(pp[nh], lhs,
                                 wo16[itile][:, nh * 512:(nh + 1) * 512],
                                 start=(itile == 0), stop=(itile == 7))
        osb = work.tile([128, DIM], F32, tag="osb", name=f"osb{tt}", bufs=2)
        nc.vector.tensor_copy(osb[:, 0:512], pp[0])
        nc.scalar.copy(osb[:, 512:1024], pp[1])
        nc.sync.dma_start(out=out_d[tt * 128:(tt + 1) * 128, :], in_=osb)


_NC_CACHE = {}


def _get_nc():
    if "nc" not in _NC_CACHE:
        _NC_CACHE["nc"] = build_kernel()
    return _NC_CACHE["nc"]


def _run(inputs, trace=False):
    x = np.ascontiguousarray(np.asarray(inputs["x"], dtype=np.float32))
    h = np.ascontiguousarray(np.asarray(inputs["h"], dtype=np.float32))
    wqkv = np.ascontiguousarray(np.asarray(inputs["Wqkv"], dtype=np.float32))
    wkr = np.ascontiguousarray(np.asarray(inputs["Wkr"], dtype=np.float32))
    r = np.ascontiguousarray(np.asarray(inputs["R"], dtype=np.float32))
    u = np.asarray(inputs["u"], dtype=np.float32)
    v = np.asarray(inputs["v"], dtype=np.float32)
    wout = np.ascontiguousarray(np.asarray(inputs["Wout"], dtype=np.float32))
    uu = np.ascontiguousarray(np.tile(u, 2).reshape(128, 1))
    vv = np.ascontiguousarray(np.tile(v, 2).reshape(128, 1))

    nc = _get_nc()
    in_maps = [
        {"x": x[b], "h": h[b], "Wqkv": wqkv, "Wkr": wkr, "R": r,
         "uu": uu, "vv": vv, "Wout": wout}
        for b in range(B)
    ]
    res = bass_utils.run_bass_kernel_spmd(
        nc, in_maps, core_ids=list(range(B)), trace=trace)
    out = np.stack([res.results[b]["out"] for b in range(B)])
    return out.astype(np.float32), res


def kernel(**inputs):
    out, _ = _run(inputs, trace=False)
    return out


# revision 28
# speedup vs baseline: 1.5956x; 1.0687x over previous
# Transformer-XL style relative-position attention on 8 Trainium2 NeuronCores.
#
# Contract: kernel(**inputs) takes the FULL unsharded inputs and returns the
# FULL [8, 256, 1024] output. Internally shards data-parallel over batch:
# core b computes batch element b. No collectives needed.
#
# Math (per batch element):
#   cat = [h; x]                            [512, 1024]
#   q,k,v = split(cat @ Wqkv)               heads=16, dhead=64
#   RW    = R @ Wkr                         [1024, 1024] (relative pos keys)
#   dots  = (q+u) @ k^T + rel_shift((q+v) @ RW_h^T)
#   out   = softmax(dots*8^-1 + causal/mem band mask) @ v @ Wout
#
# Key design points (v2 — restructured for load/compute overlap):
#  * Only 257 rows of RW are ever needed (offsets s in [256, 512]).
#  * rel_shift via DRAM scratch shear: write the [128, 258] BD band to a
#    [128, 767] scratch row and read back with AP [[766, 128], [1, 384]],
#    which realizes band[i, j] = BD[i, j - i + c]. Scratch pre-filled with
#    NEG outside the band so the read returns band+mask in one tensor.
#  * The whole schedule is arranged around the HBM load stream (24 MB of
#    f32 weights at ~370 GB/s ~= 62 us): projections run dt-outer in
#    ft-pairs so they track the arrival of each weight tile; the 32 BD
#    matmuls + band writes flood the PE/sync queues while Wk streams in;
#    the A/exp/transpose pipeline runs while Wv/Wout stream; AV + out
#    projection form the tail.
#  * Band writes and reads are split across the two HWDGE rings (sync and
#    scalar) to halve per-ring serialization.
#  * Attention inner loop is software-pipelined with explicit lookahead so
#    no engine FIFO head-blocks on the DRAM shear round trip.
#  * Normalization 1/S rides the mandatory exp transpose via diag(1/S).
#  * All matmul operands fp16; psum f32 except the exp transposes (f16).

import numpy as np

import concourse.bass as bass
import concourse.mybir as mybir
import concourse.tile as tile
from concourse import bacc, bass_utils
from concourse.masks import make_identity
from concourse.tile import add_dep_helper
from contextlib import ExitStack

F32 = mybir.dt.float32
F16 = mybir.dt.float16
AF = mybir.ActivationFunctionType

DIM = 1024
HEADS = 16
DHEAD = 64
B = 8
N = 256          # query tokens (x)
M = 256          # memory tokens (h)
T = M + N        # 512 keys
INNER = HEADS * DHEAD
SCALE = DHEAD ** -0.5
NEG = -30000.0   # fp16-representable; *0.125 still underflows exp
SW = 767         # BDs scratch width (relative offsets s = 1..767)
VAL0 = 255       # scratch col of first valid offset (s = 256)
NVALID = 257     # valid offsets s in [256, 512]
NV2 = 258        # band write width (one NEG pad col keeps mask intact)
WIN = 384        # per-query-block live key window (3 of 4 key tiles)
NBUF = 32        # BDs scratch buffers (one per iteration: no reuse, no WAR)
NIT = 32         # attention iterations (16 heads x 2 query blocks)
LAG = 4          # software-pipeline lookahead in the attention loop


def build_kernel():
    nc = bacc.Bacc("TRN2", target_bir_lowering=False, debug=False)

    x_d = nc.dram_tensor("x", [N, DIM], F32, kind="ExternalInput")
    h_d = nc.dram_tensor("h", [M, DIM], F32, kind="ExternalInput")
    wqkv_d = nc.dram_tensor("Wqkv", [DIM, 3 * INNER], F32, kind="ExternalInput")
    wkr_d = nc.dram_tensor("Wkr", [DIM, INNER], F32, kind="ExternalInput")
    r_d = nc.dram_tensor("R", [2 * T, DIM], F32, kind="ExternalInput")
    uu_d = nc.dram_tensor("uu", [128, 1], F32, kind="ExternalInput")
    vv_d = nc.dram_tensor("vv", [128, 1], F32, kind="ExternalInput")
    wout_d = nc.dram_tensor("Wout", [INNER, DIM], F32, kind="ExternalInput")
    out_d = nc.dram_tensor("out", [N, DIM], F32, kind="ExternalOutput")
    bds_d = nc.dram_tensor("bds_scratch", [NBUF, 128, SW], F16)
    junk_d = nc.dram_tensor("warm_junk", [128, 512], F16)

    with tile.TileContext(nc) as tc, ExitStack() as ctx:
        _body(ctx, tc, x_d, h_d, wqkv_d, wkr_d, r_d, uu_d, vv_d, wout_d,
              out_d, bds_d, junk_d)

    nc.compile()
    return nc


def _body(ctx, tc, x_d, h_d, wqkv_d, wkr_d, r_d, uu_d, vv_d, wout_d, out_d,
          bds_d, junk_d):
    nc = tc.nc

    const = ctx.enter_context(tc.tile_pool(name="const", bufs=1))
    persist = ctx.enter_context(tc.tile_pool(name="persist", bufs=1))
    ldpool = ctx.enter_context(tc.tile_pool(name="ld", bufs=4))
    work = ctx.enter_context(tc.tile_pool(name="work", bufs=4))
    ps_big = ctx.enter_context(tc.tile_pool(name="ps_big", bufs=4, space="PSUM"))
    ps_sml = ctx.enter_context(tc.tile_pool(name="ps_sml", bufs=2, space="PSUM"))
    ps_pav = ctx.enter_context(tc.tile_pool(name="ps_pav", bufs=2, space="PSUM"))

    # ---------------- PE warm-up (primes the HAM clock gate) ----------------
    junk = const.tile([128, 512], F16, tag="junk", name="junk")
    nc.vector.memset(junk, 1.0)
    pwarm = ps_big.tile([128, 512], F32, tag="big", name="ps_warm")
    for wi in range(16):
        nc.tensor.matmul(pwarm, junk[:, 0:128], junk,
                         start=(wi == 0), stop=(wi == 15))
    junk2 = const.tile([128, 512], F16, tag="junk2", name="junk2")
    nc.vector.tensor_copy(junk2, pwarm)
    nc.sync.dma_start(out=junk_d[:, :], in_=junk2)

    # ---------------- constants ----------------
    ident_h = const.tile([128, 128], F16, tag="identh", name="ident_h")
    make_identity(nc, ident_h)

    uu = const.tile([128, 1], F32, tag="uu", name="uu_sb")
    vv = const.tile([128, 1], F32, tag="vv", name="vv_sb")
    nc.sync.dma_start(out=uu, in_=uu_d[:, :])
    nc.sync.dma_start(out=vv, in_=vv_d[:, :])

    # ---------------- loads (single gpsimd SWDGE queue, ordered) ----------
    # cat token order: [h (0:256) | x (256:512)]; casts f32 -> f16 in flight.
    cat16 = []
    for tt in range(4):
        t_ = ldpool.tile([128, DIM], F16, tag="xh", name=f"cat16_{tt}")
        src = h_d if tt < 2 else x_d
        nc.gpsimd.dma_start(out=t_, in_=src[(tt % 2) * 128:(tt % 2) * 128 + 128, :])
        cat16.append(t_)

    # R rows needed: offsets s=256..511 -> rows 768..1023; s=512 -> row 0
    r16 = []
    for rt in range(2):
        t_ = ldpool.tile([128, DIM], F16, tag="rn", name=f"r16_{rt}", bufs=2)
        nc.gpsimd.dma_start(out=t_, in_=r_d[768 + rt * 128:768 + (rt + 1) * 128, :])
        r16.append(t_)
    # R row 0 (offset s=512), loaded directly transposed: partition p of
    # column dt holds R[0, dt*128 + p]
    r0T = const.tile([128, 8], F16, tag="r0T", name="r0T")
    nc.gpsimd.dma_start(out=r0T,
                        in_=bass.AP(r_d[0].tensor, 0, [[1, 128], [128, 8]]))

    wq16 = [persist.tile([128, INNER], F16, tag=f"wq16_{dt}", name=f"wq16_{dt}")
            for dt in range(8)]
    for dt in range(8):
        nc.gpsimd.dma_start(out=wq16[dt],
                            in_=wqkv_d[dt * 128:(dt + 1) * 128, 0:INNER])
    wkr16 = [persist.tile([128, INNER], F16, tag=f"wkr16_{dt}", name=f"wkr16_{dt}")
             for dt in range(8)]
    for dt in range(8):
        nc.gpsimd.dma_start(out=wkr16[dt], in_=wkr_d[dt * 128:(dt + 1) * 128, :])
    wk16 = [persist.tile([128, INNER], F16, tag=f"wk16_{dt}", name=f"wk16_{dt}")
            for dt in range(8)]
    for dt in range(8):
        nc.gpsimd.dma_start(out=wk16[dt],
                            in_=wqkv_d[dt * 128:(dt + 1) * 128, INNER:2 * INNER])
    wv16 = [persist.tile([128, INNER], F16, tag=f"wv16_{dt}", name=f"wv16_{dt}")
            for dt in range(8)]
    for dt in range(8):
        nc.gpsimd.dma_start(out=wv16[dt],
                            in_=wqkv_d[dt * 128:(dt + 1) * 128,
                                       2 * INNER:3 * INNER])
    wo16 = [persist.tile([128, DIM], F16, tag=f"wo16_{dt}", name=f"wo16_{dt}")
            for dt in range(8)]
    for dt in range(8):
        nc.gpsimd.dma_start(out=wo16[dt], in_=wout_d[dt * 128:(dt + 1) * 128, :])

    # ---------------- transposes of cat and R ----------------
    catT = [persist.tile([128, T], F16, tag=f"catT{dt}", name=f"catT{dt}")
            for dt in range(8)]
    for tt in range(4):
        for dt in range(8):
            pool = ps_sml if dt % 2 == 0 else ps_pav
            tp = pool.tile([128, 128], F16, tag="tp" if pool is ps_sml else "pav", name=f"tp_cat{tt}_{dt}")
            nc.tensor.transpose(tp, cat16[tt][:, dt * 128:(dt + 1) * 128],
                                ident_h)
            nc.vector.tensor_copy(catT[dt][:, tt * 128:(tt + 1) * 128], tp)

    rsubT = [persist.tile([128, NV2], F16, tag=f"rsubT{dt}", name=f"rsubT{dt}")
             for dt in range(8)]
    for rt in range(2):
        for dt in range(8):
            pool = ps_sml if dt % 2 == 0 else ps_pav
            tp = pool.tile([128, 128], F16, tag="tp" if pool is ps_sml else "pav", name=f"tp_r{rt}_{dt}")
            nc.tensor.transpose(tp, r16[rt][:, dt * 128:(dt + 1) * 128],
                                ident_h)
            nc.scalar.copy(rsubT[dt][:, rt * 128:(rt + 1) * 128], tp)
    for dt in range(8):
        nc.vector.tensor_copy(rsubT[dt][:, 256:257], r0T[:, dt:dt + 1])
        nc.vector.memset(rsubT[dt][:, 257:258], 0.0)

    # ---------------- q projection (dt-outer in ft pairs) ----------------
    # tracks the Wq load stream: the dt loop is outermost so each weight tile
    # is consumed as it arrives instead of waiting for the full matrix.
    quT = [persist.tile([128, N], F16, tag=f"quT{ft}", name=f"quT{ft}")
           for ft in range(8)]
    qvT = [persist.tile([128, N], F16, tag=f"qvT{ft}", name=f"qvT{ft}")
           for ft in range(8)]
    for g in range(4):
        qpool, qtag = (ps_big, "big") if g % 2 == 0 else (ps_pav, "pav")
        pq = [qpool.tile([128, N], F32, tag=qtag, name=f"ps_q{g}_{j}")
              for j in range(2)]
        for dt in range(8):
            for j in range(2):
                ft = 2 * g + j
                nc.tensor.matmul(pq[j], wq16[dt][:, ft * 128:(ft + 1) * 128],
                                 catT[dt][:, M:T], start=(dt == 0),
                                 stop=(dt == 7))
        for j in range(2):
            ft = 2 * g + j
            nc.vector.tensor_scalar_add(quT[ft], pq[j], uu)
            nc.vector.tensor_scalar_add(qvT[ft], pq[j], vv)

    # ---------------- RWs projection (dt-outer in ft pairs) ----------------
    rwsT = [persist.tile([128, NV2], F16, tag=f"rwsT{ft}", name=f"rwsT{ft}")
            for ft in range(8)]
    for g in range(4):
        rpool, rtag = (ps_big, "big") if g % 2 == 0 else (ps_pav, "pav")
        pr = [rpool.tile([128, NV2], F32, tag=rtag, name=f"ps_rw{g}_{j}")
              for j in range(2)]
        for dt in range(8):
            for j in range(2):
                ft = 2 * g + j
                nc.tensor.matmul(pr[j], wkr16[dt][:, ft * 128:(ft + 1) * 128],
                                 rsubT[dt], start=(dt == 0), stop=(dt == 7))
        for j in range(2):
            nc.scalar.copy(rwsT[2 * g + j], pr[j])

    # ---------------- k projection (dt-outer in ft pairs) ----------------
    kT = [persist.tile([128, T], F16, tag=f"kT{ft}", name=f"kT{ft}")
          for ft in range(8)]
    for g in range(4):
        kpool, ktag = (ps_big, "big") if g % 2 == 0 else (ps_pav, "pav")
        pk = [kpool.tile([128, T], F32, tag=ktag, name=f"ps_k{g}_{j}")
              for j in range(2)]
        for dt in range(8):
            for j in range(2):
                ft = 2 * g + j
                nc.tensor.matmul(pk[j], wk16[dt][:, ft * 128:(ft + 1) * 128],
                                 catT[dt], start=(dt == 0), stop=(dt == 7))
        for j in range(2):
            nc.vector.tensor_copy(kT[2 * g + j], pk[j])

    # ---------------- fused BD + attention pipeline ----------------
    # Stages, offset in pipeline steps (1 step = 1 iteration = head x qb):
    #   bd(s):    BD matmul + band staging into a 4-iteration batch tile;
    #             one [128, 2048] write DMA per 4 iterations covers the full
    #             shear-read window incl. NEG mask columns (no scratch
    #             pre-init, few ring DMAs - the HWDGE ring is ~1/1.1us).
    #   a(s-4):   A matmul; one batched band read DMA per 4 iterations.
    #   mid(s-6): dots = pa + band (drains pa psum), exp + rowsum.
    #   back(s-8): 1/S normalize, 3 transposes into the head's psum bank.
    # Value projection is interleaved once Wv has streamed in.
    val = [persist.tile([128, INNER], F16, tag=f"val{tt}", name=f"val{tt}")
           for tt in range(4)]
    attnT = {}   # (hh, jt) -> tile
    tpt = {}     # hh -> packed psum tile
    attn_outT = [persist.tile([128, N], F16, tag=f"aoT{ft}", name=f"aoT{ft}")
                 for ft in range(8)]

    pa_t = [None] * NIT
    band_t = [None] * NIT
    expt_t = [None] * NIT
    w_insts = [None] * (NIT // 4)
    ncopy = 0

    # band staging slots: NEG mask pads are constant, written once here;
    # per-iteration work is only the 257-column band cast.
    bsb_slots = [persist.tile([128, 2048], F16, tag=f"bsbS{j}",
                              name=f"bsbS{j}") for j in range(3)]
    for j in range(3):
        for seg in range(4):
            o = seg * 512
            nc.vector.memset(bsb_slots[j][:, o:o + 127], NEG)
            nc.vector.memset(bsb_slots[j][:, o + 384:o + 512], NEG)

    def bd(it):
        hh, qb = it // 2, it % 2
        ft, ro = hh // 2, (hh % 2) * 64
        qsl = slice(qb * 128, (qb + 1) * 128)
        b = it // 4
        bsb = bsb_slots[b % 3]
        pb = ps_big.tile([128, NV2], F32, tag="big", name=f"ps_b{it}")
        nc.tensor.matmul(pb, qvT[ft][ro:ro + 64, qsl],
                         rwsT[ft][ro:ro + 64, :], start=True, stop=True)
        o = (it % 4) * 512
        nc.vector.tensor_copy(bsb[:, o + 127:o + 127 + NVALID],
                              pb[:, 0:NVALID])
        if it % 4 == 3:
            # scratch bufs [4b, 4b+4), cols [128, 640)
            dst = bass.AP(bds_d[0].tensor, 4 * b * 128 * SW + 128,
                          [[SW, 128], [128 * SW, 4], [1, 512]])
            eng = nc.sync if b % 2 == 0 else nc.scalar
            w_insts[b] = eng.dma_start(out=dst, in_=bsb)

    def front(it):
        hh, qb = it // 2, it % 2
        ft, ro = hh // 2, (hh % 2) * 64
        qsl = slice(qb * 128, (qb + 1) * 128)
        if it % 4 == 0:
            b = it // 4
            band4 = work.tile([128, 4 * WIN], F16, tag="band",
                              name=f"band{b}", bufs=4)
            src = bass.AP(bds_d[0].tensor, 4 * b * 128 * SW + VAL0,
                          [[SW - 1, 128], [128 * SW, 4], [1, WIN]])
            eng = nc.scalar if b % 2 == 0 else nc.sync
            r_inst = eng.dma_start(out=band4, in_=src)
            add_dep_helper(r_inst.ins, w_insts[b].ins, sync=True,
                           reason="band RAW on scratch")
            for j in range(4):
                band_t[4 * b + j] = band4[:, j * WIN:(j + 1) * WIN]
        pa = ps_big.tile([128, WIN], F32, tag="big", name=f"ps_a{it}")
        nc.tensor.matmul(pa, quT[ft][ro:ro + 64, qsl],
                         kT[ft][ro:ro + 64, qb * 128:qb * 128 + WIN],
                         start=True, stop=True)
        pa_t[it] = pa

    def mid(it):
        dots = work.tile([128, WIN], F16, tag="dots", name=f"dots{it}", bufs=5)
        nc.vector.tensor_add(dots, pa_t[it], band_t[it])
        expt = work.tile([128, WIN], F16, tag="expt", name=f"expt{it}", bufs=5)
        ssum = work.tile([128, 1], F32, tag="ssum", name=f"ssum{it}", bufs=6)
        nc.scalar.activation(expt, dots, AF.Exp, bias=0.0, scale=SCALE,
                             accum_out=ssum)
        expt_t[it] = (expt, ssum)

    def back(it):
        nonlocal ncopy
        hh, qb = it // 2, it % 2
        expt, ssum = expt_t[it]
        rcp = work.tile([128, 1], F32, tag="rcp", name=f"rcp{it}", bufs=6)
        nc.vector.reciprocal(rcp, ssum)
        expn = work.tile([128, WIN], F16, tag="expn", name=f"expn{it}",
                         bufs=6)
        nc.vector.tensor_scalar_mul(expn, expt, rcp)
        # per-head packed psum layout (one bank):
        #   [jt0: 0:128 | jt1: 128:384 | jt2: 384:640 | jt3: 640:768]
        # within jt1/jt2, qb0 occupies the first 128 cols, qb1 the second.
        if hh not in tpt:
            tpt[hh] = ps_sml.tile([128, 1024], F16, tag="tp",
                                  name=f"tpt{hh}")
        offs = ([0, 128, 384] if qb == 0 else [256, 512, 640])
        for w in range(3):
            nc.tensor.transpose(tpt[hh][:, offs[w]:offs[w] + 128],
                                expn[:, w * 128:(w + 1) * 128], ident_h)
        if qb == 1:
            # drain the head's 4 key-major attention tiles
            for jt, lo, width in ((0, 0, 128), (1, 128, 256),
                                  (2, 384, 256), (3, 640, 128)):
                at = persist.tile([128, width], F16, tag=f"at{hh}_{jt}",
                                  name=f"attnT{hh}_{jt}")
                src = tpt[hh][:, lo:lo + width]
                if ncopy % 2 == 0:
                    nc.vector.tensor_copy(at, src)
                else:
                    nc.scalar.copy(at, src)
                ncopy += 1
                attnT[(hh, jt)] = at

    def vproj(tt):
        # value projection for token tile tt (needs full Wv)
        vpool, vtag = (ps_big, "big") if tt % 2 == 0 else (ps_pav, "pav")
        pv = [vpool.tile([128, 512], F32, tag=vtag, name=f"ps_v{tt}_{nh}")
              for nh in range(2)]
        for dt in range(8):
            lhs = catT[dt][:, tt * 128:(tt + 1) * 128]
            for nh in range(2):
                nc.tensor.matmul(pv[nh], lhs,
                                 wv16[dt][:, nh * 512:(nh + 1) * 512],
                                 start=(dt == 0), stop=(dt == 7))
        nc.vector.tensor_copy(val[tt][:, 0:512], pv[0])
        nc.scalar.copy(val[tt][:, 512:1024], pv[1])

    vp_after = {13: 0, 18: 1, 23: 2, 28: 3}
    for s in range(NIT + 8):
        if s < NIT:
            bd(s)
        if 0 <= s - 4 < NIT:
            front(s - 4)
        if 0 <= s - 6 < NIT:
            mid(s - 6)
        if 0 <= s - 8 < NIT:
            back(s - 8)
        if s in vp_after:
            vproj(vp_after[s])

    # ---------------- AV + output head ----------------
    for hh in range(HEADS):
        ft, ro = hh // 2, (hh % 2) * 64
        pav = ps_pav.tile([64, N], F32, tag="pav", name=f"ps_av{hh}")
        # left half (queries 0:128): keys jt 0,1,2
        nc.tensor.matmul(pav[:, 0:128], val[0][:, hh * 64:hh * 64 + 64],
                         attnT[(hh, 0)], start=True, stop=False,
                         skip_group_check=True)
        nc.tensor.matmul(pav[:, 0:128], val[1][:, hh * 64:hh * 64 + 64],
                         attnT[(hh, 1)][:, 0:128], start=False, stop=False,
                         skip_group_check=True)
        nc.tensor.matmul(pav[:, 0:128], val[2][:, hh * 64:hh * 64 + 64],
                         attnT[(hh, 2)][:, 0:128], start=False, stop=True,
                         skip_group_check=True)
        # right half (queries 128:256): keys jt 1,2,3
        nc.tensor.matmul(pav[:, 128:256], val[1][:, hh * 64:hh * 64 + 64],
                         attnT[(hh, 1)][:, 128:256], start=True, stop=False,
                         skip_group_check=True)
        nc.tensor.matmul(pav[:, 128:256], val[2][:, hh * 64:hh * 64 + 64],
                         attnT[(hh, 2)][:, 128:256], start=False, stop=False,
                         skip_group_check=True)
        nc.tensor.matmul(pav[:, 128:256], val[3][:, hh * 64:hh * 64 + 64],
                         attnT[(hh, 3)], start=False, stop=True,
                         skip_group_check=True)
        if hh % 2 == 0:
            nc.vector.tensor_copy(attn_outT[ft][ro:ro + 64, :], pav)
        else:
            nc.scalar.copy(attn_outT[ft][ro:ro + 64, :], pav)

    # ---------------- output projection ----------------
    for tt in range(2):
        pp = [ps_big.tile([128, 512], F32, tag="big", name=f"ps_o{tt}_{nh}")
              for nh in range(2)]
        for itile in range(8):
            lhs = attn_outT[itile][:, tt * 128:(tt + 1) * 128]
            for nh in range(2):
                nc.tensor.matmul(pp[nh], lhs,
                                 wo16[itile][:, nh * 512:(nh + 1) * 512],
                                 start=(itile == 0), stop=(itile == 7))
        osb = work.tile([128, DIM], F32, tag="osb", name=f"osb{tt}", bufs=2)
        nc.vector.tensor_copy(osb[:, 0:512], pp[0])
        nc.scalar.copy(osb[:, 512:1024], pp[1])
        nc.sync.dma_start(out=out_d[tt * 128:(tt + 1) * 128, :], in_=osb)


_NC_CACHE = {}


def _get_nc():
    if "nc" not in _NC_CACHE:
        _NC_CACHE["nc"] = build_kernel()
    return _NC_CACHE["nc"]


def _run(inputs, trace=False):
    x = np.ascontiguousarray(np.asarray(inputs["x"], dtype=np.float32))
    h = np.ascontiguousarray(np.asarray(inputs["h"], dtype=np.float32))
    wqkv = np.ascontiguousarray(np.asarray(inputs["Wqkv"], dtype=np.float32))
    wkr = np.ascontiguousarray(np.asarray(inputs["Wkr"], dtype=np.float32))
    r = np.ascontiguousarray(np.asarray(inputs["R"], dtype=np.float32))
    u = np.asarray(inputs["u"], dtype=np.float32)
    v = np.asarray(inputs["v"], dtype=np.float32)
    wout = np.ascontiguousarray(np.asarray(inputs["Wout"], dtype=np.float32))
    uu = np.ascontiguousarray(np.tile(u, 2).reshape(128, 1))
    vv = np.ascontiguousarray(np.tile(v, 2).reshape(128, 1))

    nc = _get_nc()
    in_maps = [
        {"x": x[b], "h": h[b], "Wqkv": wqkv, "Wkr": wkr, "R": r,
         "uu": uu, "vv": vv, "Wout": wout}
        for b in range(B)
    ]
    res = bass_utils.run_bass_kernel_spmd(
        nc, in_maps, core_ids=list(range(B)), trace=trace)
    out = np.stack([res.results[b]["out"] for b in range(B)])
    return out.astype(np.float32), res


def kernel(**inputs):
    out, _ = _run(inputs, trace=False)
    return out
